# revision 7
# baseline (speedup 1.0000x reference)
"""Trainium2 Bass kernel for nn_DAWN_10419590660472 (moe_routing transformer).

Sharding: 8 cores = 4 batches x 2 vocab-halves. Each core computes the full
4-layer body for its batch, then the tied-embedding head for all 1024 tokens
over its 16000-entry vocab half. All cores run the SAME program; only input
data differs. Heavy matmuls in bf16 with fp32 PSUM accumulation.

Perf notes (v2): the TRN2 PE clock ramps 0.65->1.2->2.4 GHz and resets on
idle, so the whole kernel is structured to keep the tensor queue fed:
- per-layer weights (incl. FFN + basis mats) prefetched at layer top through
  rotating pools so DMAs overlap prior-layer compute
- LayerNorms interleaved into the producing loops (ao-proj / FFN-down) so
  their vector chains hide under matmuls; 4 transposes share one PSUM bank
  with a single strided copy out
- routing / xA / v_sem emitted per token tile so the vector+gpsimd weighted
  accumulation of basis projections pipelines against the PE
- xA accumulation split across vector AND gpsimd engines (two partial sums)
- attention software-pipelined one head deep (scores(h) emitted before
  AV(h-1)); softmax denominators batched: one reciprocal per query chunk
- head: embedding loads (sync queue) decoupled from logit stores (scalar
  queue) so streaming prefetch is never blocked

Host-folded math (unchanged): attn.mean(-1)==1/S so the routing gate is a
per-layer constant folded into the sem projection; LN affines folded into
downstream weights; top_k via nc.vector.max + masked softmax as a matmul;
attention denominators via a ones-augmented value matrix.
"""

import numpy as np
import ml_dtypes

VOC = 32000; D = 512; DFF = 2048; L = 4; H = 8; DH = D // H
NN = 256; NB = 32; R = 128; B = 4; S = 1024
NCORES = 8
TT = S // 128          # 8 token tiles
DS = D // 128          # 4 d-slices
FS = DFF // 128        # 16 dff-slices
QC = S // 512          # 2 query chunks
VH = VOC // 2          # vocab half per core
VCH = 500              # head vocab chunk (<=512)
NVC = VH // VCH        # 32
VG = 8                 # emb streaming groups
VPG = NVC // VG        # 4 chunks per group
EPS = 1e-5

BF16 = ml_dtypes.bfloat16
_cache = {}


def _softmax_np(x, axis=-1):
    m = x.max(axis=axis, keepdims=True)
    e = np.exp(x - m)
    return e / e.sum(axis=axis, keepdims=True)


def _preprocess(inputs):
    f32 = lambda k: np.asarray(inputs[k], dtype=np.float32)
    ids = np.asarray(inputs["input_ids"])
    token_emb = f32("token_emb"); pos_emb = f32("pos_emb")
    basis_A = f32("basis_A"); basis_emb = f32("basis_emb")
    q_w = f32("q_w"); k_w = f32("k_w"); ao_w = f32("ao_w")
    recipe = f32("recipe"); ctx_pat = f32("ctx_pat")
    vout_w = f32("vout_w"); up_w = f32("up_w"); down_w = f32("down_w")
    ln1_s = f32("ln1_s"); ln2_s = f32("ln2_s"); lnf_s = f32("lnf_s")

    for k in ("q_b", "k_b", "ao_b", "vout_b", "up_b", "down_b",
              "ln1_b", "ln2_b", "lnf_b"):
        assert not np.any(np.asarray(inputs[k])), f"nonzero {k} unsupported"

    scale = 1.0 / np.sqrt(DH)
    x0 = token_emb[ids] + pos_emb[:S][None]              # [B, S, D]

    def part_first(a, nslice):
        # [nslice*128, F] -> [128, nslice, F]
        return np.ascontiguousarray(
            a.reshape(nslice, 128, -1).transpose(1, 0, 2))

    wq = np.empty((L, 128, DS, D), dtype=BF16)
    wk = np.empty((L, 128, DS, D), dtype=BF16)
    wao = np.empty((L, 128, DS, D), dtype=BF16)
    gT = np.empty((L, 128, DS, NN), dtype=BF16)
    recT = np.empty((L, 128, 2, NB), dtype=BF16)
    a_cat = np.empty((L, 128, DS, NB * R), dtype=BF16)
    wvout = np.empty((L, 128, D), dtype=BF16)
    wup = np.empty((L, 128, DS, DFF), dtype=BF16)
    wdn = np.empty((L, 128, FS, D), dtype=BF16)

    for l in range(L):
        wq[l] = part_first((q_w[l] * ln1_s[l][None, :] * scale).T, DS)
        wk[l] = part_first((k_w[l] * ln1_s[l][None, :]).T, DS)
        wao[l] = part_first(ao_w[l].T, DS)
        rs = _softmax_np(recipe[l])                      # [NN, NB]
        emb_sem = rs @ basis_emb                         # [NN, D]
        gate = 1.0 / (1.0 + np.exp(-(ctx_pat[l].sum(-1) / S)))
        gT[l] = part_first(((emb_sem * ln1_s[l][None, :]) * gate[:, None]).T, DS)
        recT[l] = part_first(rs, 2)
        ae = basis_A * ln1_s[l][None, :, None]           # [NB, D, R]
        a_cat[l] = part_first(ae.transpose(1, 0, 2).reshape(D, NB * R), DS)
        wvout[l] = vout_w[l].T.astype(BF16)              # [R, D]
        wup[l] = part_first((up_w[l] * ln2_s[l][None, :]).T, DS)
        wdn[l] = part_first(down_w[l].T, FS)

    eT_full = part_first((token_emb * lnf_s[None, :]).T, DS).astype(BF16)
    ident = np.eye(128, dtype=BF16)

    shared = dict(wq=wq, wk=wk, wao=wao, gT=gT, recT=recT, a_cat=a_cat,
                  wvout=wvout, wup=wup, wdn=wdn, ident=ident)
    per_core = []
    for c in range(NCORES):
        b, half = c // 2, c % 2
        m = dict(shared)
        m["x0"] = np.ascontiguousarray(x0[b]).astype(np.float32)
        m["eT"] = np.ascontiguousarray(eT_full[:, :, half * VH:(half + 1) * VH])
        per_core.append(m)
    return per_core


def _build_nc():
    import concourse.mybir as mybir
    import concourse.tile as tile
    from concourse import bacc
    from concourse.alu_op_type import AluOpType as Alu

    AF = mybir.ActivationFunctionType
    bf = mybir.dt.bfloat16
    f32 = mybir.dt.float32

    nc = bacc.Bacc("TRN2", target_bir_lowering=False, debug=False,
                   num_devices=NCORES)

    din = lambda n, shp, dt=bf: nc.dram_tensor(n, shp, dt, kind="ExternalInput")
    dr = dict(
        x0=din("x0", [S, D], f32),
        wq=din("wq", [L, 128, DS, D]), wk=din("wk", [L, 128, DS, D]),
        wao=din("wao", [L, 128, DS, D]), gT=din("gT", [L, 128, DS, NN]),
        recT=din("recT", [L, 128, 2, NB]),
        a_cat=din("a_cat", [L, 128, DS, NB * R]),
        wvout=din("wvout", [L, 128, D]), wup=din("wup", [L, 128, DS, DFF]),
        wdn=din("wdn", [L, 128, FS, D]), eT=din("eT", [128, DS, VH]),
        ident=din("ident", [128, 128]),
        out=nc.dram_tensor("logits", [S, VH], f32, kind="ExternalOutput"),
    )

    with tile.TileContext(nc) as tc:
        _emit(nc, tc, mybir, Alu, AF, bf, f32, dr)

    nc.compile()
    return nc


def _emit(nc, tc, mybir, Alu, AF, bf, f32, dr):
    from contextlib import ExitStack
    ctx = ExitStack()
    pool = lambda name, bufs, space="SBUF": ctx.enter_context(
        tc.tile_pool(name=name, bufs=bufs, space=space))

    P_x = pool("x", 1)
    P_const = pool("const", 1)
    P_w = pool("w", 1)                 # small per-layer weights
    P_big = pool("big", 2)             # a_cat halves / wup / wdn / emb chunks
    P_act = pool("act", 1)             # per-layer activations
    P_nrm = pool("nrm", 1)             # token-major LN outputs (transient)
    P_attn = pool("attn", 2)           # eT buffers
    P_rt = pool("rt", 2)               # routing temporaries
    P_sm = pool("sm", 2)               # small stats tiles
    P_ao = pool("ao", 1)               # unnormalized AV outputs (8 heads)
    P_at2 = pool("at2", 1)             # attention denominator tiles
    P_hd = pool("hd", 2)               # head staging
    P_ps = pool("ps", 2, "PSUM")       # generic matmul psum
    P_psT = pool("psT", 1, "PSUM")     # transpose psum (4x128 batched)
    P_psA = pool("psA", 2, "PSUM")     # xA psum
    P_pss = pool("pss", 2, "PSUM")     # attention scores psum
    P_psa = pool("psa", 1, "PSUM")     # attention AV psum

    ident = P_const.tile([128, 128], bf)
    nc.sync.dma_start(out=ident, in_=dr["ident"][:, :])
    eps_sb = P_const.tile([128, 1], f32)
    nc.vector.memset(eps_sb, EPS)

    x_sb = P_x.tile([128, TT, D], f32)
    for t in range(TT):
        nc.sync.dma_start(out=x_sb[:, t, :],
                          in_=dr["x0"][t * 128:(t + 1) * 128, :])

    def layernorm(src_ap, dst_bf):
        stats = P_sm.tile([128, 6], f32, tag="st")
        nc.vector.bn_stats(out=stats, in_=src_ap)
        mv = P_sm.tile([128, 2], f32, tag="mv")
        nc.vector.bn_aggr(out=mv, in_=stats)
        rstd = P_sm.tile([128, 1], f32, tag="rs")
        nc.scalar.activation(out=rstd, in_=mv[:, 1:2], func=AF.Sqrt,
                             bias=eps_sb)
        nc.vector.reciprocal(out=rstd, in_=rstd)
        nc.vector.tensor_scalar(out=dst_bf, in0=src_ap, scalar1=mv[:, 0:1],
                                scalar2=rstd, op0=Alu.subtract, op1=Alu.mult)

    def transpose512(dstT, t, src_bf):
        # src [128 tok, 512 d] -> dstT[:, ds, t*128:(t+1)*128] for ds in 0..3
        # 4 transposes share one PSUM tile; one strided copy drains it.
        ps = P_psT.tile([128, 512], bf, tag="psT")
        for j in range(DS):
            nc.tensor.transpose(ps[:, j * 128:(j + 1) * 128],
                                src_bf[:, j * 128:(j + 1) * 128], ident)
        nc.scalar.copy(out=dstT[:, :, t * 128:(t + 1) * 128],
                       in_=ps.rearrange("p (a b) -> p a b", a=DS))

    def transpose128(dst_sb, src_sb):
        ps = P_psT.tile([128, 512], bf, tag="psT")
        nc.tensor.transpose(ps[:, 0:128], src_sb, ident)
        nc.scalar.copy(out=dst_sb, in_=ps[:, 0:128])

    def ln_transpose(t, dstT):
        nrm = P_nrm.tile([128, D], bf, tag="nrm")
        layernorm(x_sb[:, t, :], nrm)
        transpose512(dstT, t, nrm)

    # LN1 for layer 0 (later layers fold into previous FFN-down loop)
    nrmT = P_act.tile([128, DS, S], bf, tag="nrmT")
    for t in range(TT):
        ln_transpose(t, nrmT)

    for l in range(L):
        wq_l = P_w.tile([128, DS, D], bf, tag="wq")
        wk_l = P_w.tile([128, DS, D], bf, tag="wk")
        wao_l = P_w.tile([128, DS, D], bf, tag="wao")
        g_l = P_w.tile([128, DS, NN], bf, tag="g")
        rec_l = P_w.tile([128, 2, NB], bf, tag="rec")
        wv_l = P_w.tile([128, D], bf, tag="wv")
        nc.sync.dma_start(out=wq_l, in_=dr["wq"][l])
        nc.sync.dma_start(out=wk_l, in_=dr["wk"][l])
        nc.sync.dma_start(out=wao_l, in_=dr["wao"][l])
        nc.sync.dma_start(out=g_l, in_=dr["gT"][l])
        nc.sync.dma_start(out=rec_l, in_=dr["recT"][l])
        nc.sync.dma_start(out=wv_l, in_=dr["wvout"][l])
        # basis mats + FFN weights rotate through P_big; DMAs overlap compute
        a0_l = P_big.tile([128, DS, NB * R // 2], bf, tag="big")
        a1_l = P_big.tile([128, DS, NB * R // 2], bf, tag="big")
        nc.sync.dma_start(out=a0_l, in_=dr["a_cat"][l][:, :, :NB * R // 2])
        nc.sync.dma_start(out=a1_l, in_=dr["a_cat"][l][:, :, NB * R // 2:])
        wup_l = P_big.tile([128, DS, DFF], bf, tag="big")
        wdn_l = P_big.tile([128, FS, D], bf, tag="big")
        nc.sync.dma_start(out=wup_l, in_=dr["wup"][l])
        nc.sync.dma_start(out=wdn_l, in_=dr["wdn"][l])

        qT = P_act.tile([128, DS, S], bf, tag="qT")  # slot reused by FFN hT
        kT = P_act.tile([128, DS, S], bf, tag="kT")
        vv = P_act.tile([128, TT, H * (DH + 1)], bf, tag="vv")
        aoT = P_act.tile([128, DS, S], bf, tag="aoT")
        tr_all = P_act.tile([128, TT, NB], f32, tag="tr")
        vs_v = P_act.tile([128, TT, R], f32, tag="vsv")   # v_sem accumulator

        # ---- Q/K projections (outputs stay [d_out, tok]) ----
        for qc in range(QC):
            for ot in range(DS):
                for (w_l, dstT) in ((wq_l, qT), (wk_l, kT)):
                    ps = P_ps.tile([128, 512], f32, tag="ps")
                    for ds in range(DS):
                        nc.tensor.matmul(
                            ps, w_l[:, ds, ot * 128:(ot + 1) * 128],
                            nrmT[:, ds, qc * 512:(qc + 1) * 512],
                            start=(ds == 0), stop=(ds == DS - 1))
                    nc.scalar.copy(out=dstT[:, ot, qc * 512:(qc + 1) * 512],
                                   in_=ps)

        # ---- per token tile: routing -> xA -> v_sem -> Vv ----
        for t in range(TT):
            # routing: fin -> top8 -> masked softmax -> token_recipe
            fin_ps = P_ps.tile([128, 512], f32, tag="ps")
            for ds in range(DS):
                nc.tensor.matmul(fin_ps[:, :NN],
                                 nrmT[:, ds, t * 128:(t + 1) * 128],
                                 g_l[:, ds, :],
                                 start=(ds == 0), stop=(ds == DS - 1))
            fin = P_rt.tile([128, NN], f32, tag="fin")
            nc.scalar.copy(out=fin, in_=fin_ps[:, :NN])
            m8 = P_rt.tile([128, 8], f32, tag="m8")
            nc.vector.max(out=m8, in_=fin)
            t8 = P_sm.tile([128, 1], f32, tag="t8")
            nc.vector.reduce_sum(out=t8, in_=m8, axis=mybir.AxisListType.X,
                                 op=Alu.min)   # 8th largest, order-agnostic
            nt8 = P_sm.tile([128, 1], f32, tag="nt8")
            nc.vector.tensor_scalar_mul(out=nt8, in0=t8, scalar1=-1.0)
            er = P_rt.tile([128, NN], f32, tag="er")
            nc.scalar.activation(out=er, in_=fin, func=AF.Exp, bias=nt8)
            we = P_rt.tile([128, NN], f32, tag="we")
            nc.vector.scalar_tensor_tensor(out=we, in0=fin, scalar=t8,
                                           in1=er, op0=Alu.is_ge, op1=Alu.mult)
            dn = P_sm.tile([128, 1], f32, tag="dn")
            nc.vector.reduce_sum(out=dn, in_=we, axis=mybir.AxisListType.X)
            rc = P_sm.tile([128, 1], f32, tag="rc")
            nc.vector.reciprocal(out=rc, in_=dn)
            wfull = P_rt.tile([128, NN], bf, tag="wfull")
            nc.vector.tensor_scalar_mul(out=wfull, in0=we, scalar1=rc)
            wfT = P_rt.tile([128, 2, 128], bf, tag="wfT")
            psw = P_psT.tile([128, 512], bf, tag="psT")
            for ns in range(2):
                nc.tensor.transpose(psw[:, ns * 128:(ns + 1) * 128],
                                    wfull[:, ns * 128:(ns + 1) * 128], ident)
            nc.scalar.copy(out=wfT,
                           in_=psw[:, 0:256].rearrange("p (a b) -> p a b", a=2))
            tr_ps = P_ps.tile([128, 512], f32, tag="ps")
            for ns in range(2):
                nc.tensor.matmul(tr_ps[:, :NB], wfT[:, ns, :], rec_l[:, ns, :],
                                 start=(ns == 0), stop=(ns == 1))
            nc.vector.tensor_copy(out=tr_all[:, t, :], in_=tr_ps[:, :NB])

            # xA: 8 groups of 4 basis mats; weighted accumulation split
            # across vector (vs_v) and gpsimd (vs_g) partial sums
            for g in range(8):
                a_l = a0_l if g < 4 else a1_l
                go = g % 4
                psA = P_psA.tile([128, 512], f32, tag="psA")
                for ds in range(DS):
                    nc.tensor.matmul(psA,
                                     nrmT[:, ds, t * 128:(t + 1) * 128],
                                     a_l[:, ds, go * 512:(go + 1) * 512],
                                     start=(ds == 0), stop=(ds == DS - 1))
                # stage PSUM -> SBUF bf16 on scalar; vector then reads at
                # 2 elem/cycle, keeping the weighted accumulation off the
                # critical path of the PE (gpsimd can't run these ops)
                xa_sb = P_rt.tile([128, 512], bf, tag="xasb")
                nc.scalar.copy(out=xa_sb, in_=psA)
                for ni in range(4):
                    n = g * 4 + ni
                    if n == 0:
                        nc.vector.tensor_scalar_mul(
                            out=vs_v[:, t, :], in0=xa_sb[:, ni * R:(ni + 1) * R],
                            scalar1=tr_all[:, t, n:n + 1])
                    else:
                        nc.vector.scalar_tensor_tensor(
                            out=vs_v[:, t, :],
                            in0=xa_sb[:, ni * R:(ni + 1) * R],
                            scalar=tr_all[:, t, n:n + 1],
                            in1=vs_v[:, t, :],
                            op0=Alu.mult, op1=Alu.add)

            # v_sem = vs_v + vs_g; transpose; Vv projection
            vs_bf = P_rt.tile([128, R], bf, tag="vsbf")
            nc.vector.tensor_copy(out=vs_bf, in_=vs_v[:, t, :])
            vsTt = P_rt.tile([128, R], bf, tag="vsT")
            transpose128(vsTt, vs_bf)
            psv = P_ps.tile([128, 512], f32, tag="ps")
            nc.tensor.matmul(psv, vsTt, wv_l,
                             start=True, stop=True)
            # per-head layout [Vv_h | 1]: the ones column makes the AV matmul
            # also produce the softmax denominator (psum partition 64)
            vvh = vv[:, t, :].rearrange("p (h e) -> p h e", h=H)
            nc.scalar.copy(out=vvh[:, :, 0:DH],
                           in_=psv.rearrange("p (h e) -> p h e", h=H))
            nc.vector.memset(vvh[:, :, DH:DH + 1], 1.0)

        # ---- attention: head-pipelined; batched softmax denominators ----
        for qc in range(QC):
            nkt = qc * 4 + 4
            eT_t = {}
            aoU_t = {}
            dn8 = P_at2.tile([8, 512], f32, tag="dn8")
            for h in range(H + 1):
                if h < H:
                    hp = (h % 2) * 64
                    hd = h // 2
                    eT = P_attn.tile([128, TT, 512], bf, tag="eT")
                    eT_t[h] = eT
                    for kt in range(nkt):
                        pss = P_pss.tile([128, 512], f32, tag="pss")
                        nc.tensor.matmul(
                            pss, kT[hp:hp + 64, hd, kt * 128:(kt + 1) * 128],
                            qT[hp:hp + 64, hd, qc * 512:(qc + 1) * 512],
                            start=True, stop=True)
                        nc.scalar.activation(out=eT[:, kt, :], in_=pss,
                                             func=AF.Exp)
                        kt_rel = kt - qc * 4
                        if kt_rel >= 0:
                            if kt_rel > 0:
                                nc.vector.memset(eT[:, kt, 0:kt_rel * 128], 0.0)
                            nc.gpsimd.affine_select(
                                out=eT[:, kt, kt_rel * 128:(kt_rel + 1) * 128],
                                in_=eT[:, kt, kt_rel * 128:(kt_rel + 1) * 128],
                                compare_op=Alu.is_ge, fill=0.0, base=0,
                                pattern=[[1, 128]], channel_multiplier=-1)
                if h >= 1:
                    hh = h - 1
                    eTp = eT_t.pop(hh)
                    psa = P_psa.tile([128, 512], f32, tag="psa")
                    for kt in range(nkt):
                        nc.tensor.matmul(
                            psa[0:DH + 1, :],
                            vv[:, kt, hh * (DH + 1):(hh + 1) * (DH + 1)],
                            eTp[:, kt, :], start=(kt == 0),
                            stop=(kt == nkt - 1))
                    aoU = P_ao.tile([DH + 1, 512], bf, tag=f"aoU{hh}")
                    nc.vector.tensor_copy(out=aoU, in_=psa[0:DH + 1, :])
                    nc.gpsimd.dma_start(out=dn8[hh:hh + 1, :],
                                        in_=aoU[DH:DH + 1, :])
                    aoU_t[hh] = aoU
            rc8 = P_at2.tile([8, 512], f32, tag="rc8")
            nc.vector.reciprocal(out=rc8, in_=dn8)
            rc8f = P_at2.tile([1, H * 512], f32, tag="rc8f")
            nc.gpsimd.dma_start(out=rc8f, in_=rc8)   # partitions -> free dim
            for h in range(H):
                hp = (h % 2) * 64
                hd = h // 2
                rb = P_sm.tile([64, 512], f32, tag="rb")
                nc.gpsimd.partition_broadcast(rb, rc8f[0:1, h * 512:(h + 1) * 512])
                if hp == 0:
                    nc.vector.tensor_tensor(
                        out=aoT[0:64, hd, qc * 512:(qc + 1) * 512],
                        in0=aoU_t[h][0:DH, :], in1=rb, op=Alu.mult)
                else:
                    tmp = P_sm.tile([64, 512], bf, tag="aotmp")
                    nc.vector.tensor_tensor(out=tmp, in0=aoU_t[h][0:DH, :],
                                            in1=rb, op=Alu.mult)
                    nc.sync.dma_start(
                        out=aoT[64:128, hd, qc * 512:(qc + 1) * 512], in_=tmp)

        # ---- attention out proj + residual + LN2 (interleaved per tile) ----
        n2T = P_act.tile([128, DS, S], bf, tag="n2T")
        for t in range(TT):
            pso = P_ps.tile([128, 512], f32, tag="ps")
            for ds in range(DS):
                nc.tensor.matmul(pso, aoT[:, ds, t * 128:(t + 1) * 128],
                                 wao_l[:, ds, :],
                                 start=(ds == 0), stop=(ds == DS - 1))
            nc.vector.tensor_tensor(out=x_sb[:, t, :], in0=pso,
                                    in1=x_sb[:, t, :], op=Alu.add)
            nrm = P_nrm.tile([128, D], bf, tag="nrm")
            layernorm(x_sb[:, t, :], nrm)
            transpose512(n2T, t, nrm)

        # ---- FFN; LN1 of next layer (or final LN) folded into down loop ----
        nextT = P_act.tile([128, DS, S], bf, tag="nrmT")
        for qc in range(QC):
            hT = P_act.tile([128, FS, 512], bf, tag="qT")
            for ft in range(FS):
                psu = P_ps.tile([128, 512], f32, tag="ps")
                for ds in range(DS):
                    nc.tensor.matmul(psu,
                                     wup_l[:, ds, ft * 128:(ft + 1) * 128],
                                     n2T[:, ds, qc * 512:(qc + 1) * 512],
                                     start=(ds == 0), stop=(ds == DS - 1))
                nc.scalar.activation(out=hT[:, ft, :], in_=psu, func=AF.Gelu)
            for tr in range(4):
                t = qc * 4 + tr
                psd = P_ps.tile([128, 512], f32, tag="ps")
                for fs in range(FS):
                    nc.tensor.matmul(psd, hT[:, fs, tr * 128:(tr + 1) * 128],
                                     wdn_l[:, fs, :],
                                     start=(fs == 0), stop=(fs == FS - 1))
                nc.vector.tensor_tensor(out=x_sb[:, t, :], in0=psd,
                                        in1=x_sb[:, t, :], op=Alu.add)
                nrm2 = P_nrm.tile([128, D], bf, tag="nrm")
                layernorm(x_sb[:, t, :], nrm2)
                transpose512(nextT, t, nrm2)
        nrmT = nextT    # becomes LN1^T of next layer / final-LN^T after l=L-1

    # ---- tied head over this core's vocab half ----
    xfT = nrmT
    for vg in range(VG):
        emb = P_big.tile([128, DS, VPG * VCH], bf, tag="big")
        nc.sync.dma_start(
            out=emb, in_=dr["eT"][:, :, vg * VPG * VCH:(vg + 1) * VPG * VCH])
        for t in range(TT):
            for vi in range(VPG):
                psh = P_ps.tile([128, 512], f32, tag="ps")
                for ds in range(DS):
                    nc.tensor.matmul(
                        psh[:, :VCH], xfT[:, ds, t * 128:(t + 1) * 128],
                        emb[:, ds, vi * VCH:(vi + 1) * VCH],
                        start=(ds == 0), stop=(ds == DS - 1))
                stage = P_hd.tile([128, VCH], f32, tag="stage")
                if vi % 2 == 0:
                    nc.vector.tensor_copy(out=stage, in_=psh[:, :VCH])
                else:
                    nc.scalar.copy(out=stage, in_=psh[:, :VCH])
                off = (vg * VPG + vi) * VCH
                # logit stores issue from the scalar queue so they never
                # block the next emb-group prefetch on the sync queue
                nc.scalar.dma_start(
                    out=dr["out"][t * 128:(t + 1) * 128, off:off + VCH],
                    in_=stage)
    ctx.close()


def kernel(**inputs):
    from concourse.bass_utils import run_bass_kernel_spmd

    if "nc" not in _cache:
        _cache["nc"] = _build_nc()
    nc = _cache["nc"]

    in_maps = _preprocess(inputs)
    res = run_bass_kernel_spmd(nc, in_maps, core_ids=list(range(NCORES)))
    global _last_results
    _last_results = res.results

    out = np.empty((B, S, VOC), dtype=np.float32)
    for c in range(NCORES):
        b, half = c // 2, c % 2
        out[b, :, half * VH:(half + 1) * VH] = res.results[c]["logits"]
    return out
